# revision 18
# baseline (speedup 1.0000x reference)
"""Trainium2 Bass kernel for nn_Decoder (dense MLP).

Computes out = relu(V @ W1 + b1) @ W2 + b2 for V [262144, 1024],
W1 [1024, 128], W2 [128, 4].

Strategy
--------
Data-parallel over 8 NeuronCores: V is sharded along rows (32768 rows per
core); the small weights are replicated.

Single-pass fp16 (~4e-4 rel err, gate is 2e-2) instead of a multi-pass
hi/lo split — 3x less PE work; fp8 variants were measured at 2.9e-2+ and
fail the gate, so 2 B/elem is the DMA floor. Layer 1 computes
h.T = W1.T @ V.T via PSUM-accumulated matmuls, k-major over S=4 PSUM
banks so each W1 k-chunk's 128-col weight load is amortized over 4
matmuls. ReLU(+b1) on the scalar engine reads PSUM and emits fp16; layer
2 (out.T = W2.T @ h.T) runs on the tensor engine; b2 is added on the
vector engine and results are stored in one batched 32 KB DMA per
2048-column superchunk (small interleaved stores measurably slow the V
read stream via HBM read/write turnarounds).

The host pre-arranges each core's V shard as [nchunk, 128, kc, 512] fp16
so every 512-column chunk is one 1 MB DMA whose per-partition source AND
SBUF destination runs are a contiguous 8 KB — near-ideal descriptors
(a 1 KB-run variant measured ~10% slower). Per-chunk DMA granularity
keeps tensor-engine waits in small quanta so the PE clock stays warm
(no >3.4 us HAM idle windows).

Regime: HBM-bound. V streams at the measured ~350 GB/s per-core cap;
67 MB / 350 GB/s ~= 193 us. Measured per-core exec 195-228 us
(mean ~206 us, spread is cross-core HBM/profiling contention),
vs 563 us baseline.
"""

import os
import sys

import numpy as np

for _p in ("/opt/trn_rl_repo", "/root/.axon_site/_ro/trn_rl_repo"):
    if os.path.isdir(_p) and _p not in sys.path:
        sys.path.insert(0, _p)

import concourse.bass as bass
import concourse.mybir as mybir
import concourse.tile as tile
from concourse import bacc
from concourse.bass_utils import run_bass_kernel_spmd

NCORES = 8
NN = 262144
IN_DIM = 1024
HIDDEN = 128
OUT_DIM = 4
R = NN // NCORES  # rows per core

P = 128            # SBUF partitions
KC = IN_DIM // P   # 8 k-chunks
CHUNK = 512        # columns per PSUM accumulation tile (one PSUM bank)
S = 2              # chunks per superchunk (live PSUM accumulator banks;
                   # S=2 + 4 PSUM bufs double-buffers accumulators across
                   # superchunks, removing the ACT-wait stall at each
                   # boundary and halving the post-stream trailing compute)
SGROUP = S * CHUNK # 1024 columns per superchunk
DATA_BUFS = 8      # prefetch depth for V superchunk tiles (~16.8 MB)

_last_results = None  # exposed for test harness (exec_time_ns etc.)
MODE = "f16"


def build_nc(rows=R):
    """Build the SPMD Bass program for one core."""
    f32 = mybir.dt.float32
    f16 = mybir.dt.float16

    nc = bacc.Bacc("TRN2")

    nch = rows // CHUNK
    vt_d = nc.declare_dram_parameter("VT", [nch, P, KC, CHUNK], f16, isOutput=False)
    w1_d = nc.declare_dram_parameter("W1T", [IN_DIM, HIDDEN], f16, isOutput=False)
    b1_d = nc.declare_dram_parameter("B1", [HIDDEN, 1], f32, isOutput=False)
    w2_d = nc.declare_dram_parameter("W2T", [HIDDEN, OUT_DIM], f16, isOutput=False)
    b2_d = nc.declare_dram_parameter("B2", [OUT_DIM, 1], f32, isOutput=False)
    out_d = nc.declare_dram_parameter("OUT", [OUT_DIM, rows], f32, isOutput=True)

    nsc = rows // SGROUP

    with tile.TileContext(nc) as tc:
        with (
            tc.tile_pool(name="const", bufs=1) as cpool,
            tc.tile_pool(name="data", bufs=DATA_BUFS) as dpool,
            tc.tile_pool(name="work", bufs=3) as wpool,
            tc.tile_pool(name="psum1", bufs=4, space="PSUM") as ppool,
            tc.tile_pool(name="psum2", bufs=2, space="PSUM") as opool,
        ):
            # --- constants ---
            # W1 goes first on the sync queue (first matmul needs it);
            # the small consts ride the scalar-engine HWDGE ring so they
            # don't delay the V stream on the sync ring.
            w1_sb = cpool.tile([P, KC, HIDDEN], f16)
            nc.sync.dma_start(w1_sb[:], w1_d[:].rearrange("(c p) h -> p c h", p=P))
            b1_sb = cpool.tile([HIDDEN, 1], f32)
            nc.scalar.dma_start(b1_sb[:], b1_d[:])
            w2_sb = cpool.tile([HIDDEN, OUT_DIM], f16)
            nc.scalar.dma_start(w2_sb[:], w2_d[:])
            b2_sb = cpool.tile([OUT_DIM, 1], f32)
            nc.scalar.dma_start(b2_sb[:], b2_d[:])

            vt_view = vt_d[:]
            out_pair_view = out_d[:].rearrange("o (g n) -> g o n", n=2 * SGROUP)
            o_sb = None

            for g in range(nsc):
                v = dpool.tile([P, S, KC, CHUNK], f16, tag="v")
                # per-chunk 1 MB DMAs: source and SBUF destination are
                # both one contiguous 8 KB run per partition (ideal
                # descriptors), and PE waits come in small quanta so
                # the tensor engine never idles >3.4us (HAM stays warm)
                for j in range(S):
                    if g == 0:
                        # halve the first superchunk's transfers so the
                        # first matmul's data lands sooner
                        for h in range(2):
                            slk = slice(h * (KC // 2), (h + 1) * (KC // 2))
                            nc.sync.dma_start(
                                v[:, j, slk, :], vt_view[g * S + j][:, slk, :]
                            )
                    else:
                        nc.sync.dma_start(v[:, j, :, :], vt_view[g * S + j])

                # layer 1: k-major so each W1 k-chunk stays stationary
                # across S matmuls (amortizes the 128-col weight load)
                ps = [
                    ppool.tile([HIDDEN, CHUNK], f32, tag="ps", name=f"ps{j}")
                    for j in range(S)
                ]
                for c in range(KC):
                    for j in range(S):
                        nc.tensor.matmul(
                            ps[j][:],
                            w1_sb[:, c, :],
                            v[:, j, c, :],
                            start=(c == 0),
                            stop=(c == KC - 1),
                        )

                if g % 2 == 0:
                    o_sb = wpool.tile([OUT_DIM, 2 * SGROUP], f32, tag="o")
                for j in range(S):
                    hh = wpool.tile([HIDDEN, CHUNK], f16, tag="hh")
                    nc.scalar.activation(
                        hh[:], ps[j][:],
                        mybir.ActivationFunctionType.Relu,
                        bias=b1_sb[:],
                    )
                    po = opool.tile([OUT_DIM, CHUNK], f32, tag="po")
                    nc.tensor.matmul(po[:], w2_sb[:], hh[:], start=True, stop=True)
                    off = (g % 2) * SGROUP + j * CHUNK
                    nc.vector.tensor_scalar_add(
                        o_sb[:, off : off + CHUNK], po[:], b2_sb[:]
                    )

                # one batched 32 KB store per superchunk pair: few HBM
                # read/write turnarounds (64 small stores measurably
                # slowed the V read stream)
                if g % 2 == 1:
                    nc.scalar.dma_start(out_pair_view[g // 2], o_sb[:])

    return nc


def kernel(V, W1, b1, W2, b2):
    global _last_results

    V = np.asarray(V, dtype=np.float32)
    W1 = np.asarray(W1, dtype=np.float32)
    b1 = np.asarray(b1, dtype=np.float32)
    W2 = np.asarray(W2, dtype=np.float32)
    b2 = np.asarray(b2, dtype=np.float32)

    common = {
        "W1T": W1.astype(np.float16),
        "W2T": np.ascontiguousarray(W2).astype(np.float16),
        "B1": np.ascontiguousarray(b1.reshape(HIDDEN, 1)),
        "B2": np.ascontiguousarray(b2.reshape(OUT_DIM, 1)),
    }

    in_maps = []
    for c in range(NCORES):
        shard = V[c * R : (c + 1) * R]  # [R, IN_DIM]
        # [nchunk, 512, kc, 128] -> [nchunk, 128, kc, 512]: each chunk's
        # per-partition data is one contiguous 8 KB run
        vh = np.ascontiguousarray(
            shard.reshape(R // CHUNK, CHUNK, KC, P)
            .transpose(0, 3, 2, 1)
            .astype(np.float16)
        )
        m = {"VT": vh}
        m.update(common)
        in_maps.append(m)

    nc = build_nc(R)
    nc.finalize()
    res = run_bass_kernel_spmd(nc, in_maps, list(range(NCORES)))
    _last_results = res

    out = np.concatenate(
        [np.asarray(r["OUT"]).T for r in res.results], axis=0
    ).astype(np.float32)
    return out


# revision 20
# speedup vs baseline: 1.0241x; 1.0241x over previous
"""Trainium2 Bass kernel for nn_Decoder (dense MLP).

Computes out = relu(V @ W1 + b1) @ W2 + b2 for V [262144, 1024],
W1 [1024, 128], W2 [128, 4].

Strategy
--------
Data-parallel over 8 NeuronCores: V is sharded along rows (32768 rows per
core); the small weights are replicated.

Single-pass fp16 (~4e-4 rel err, gate is 2e-2) instead of a multi-pass
hi/lo split — 3x less PE work; fp8 variants were measured at 2.9e-2+ and
fail the gate, so 2 B/elem is the DMA floor. Layer 1 computes
h.T = W1.T @ V.T via PSUM-accumulated matmuls, k-major over S=4 PSUM
banks so each W1 k-chunk's 128-col weight load is amortized over 4
matmuls. ReLU(+b1) on the scalar engine reads PSUM and emits fp16; layer
2 (out.T = W2.T @ h.T) runs on the tensor engine; b2 is added on the
vector engine and results are stored in one batched 32 KB DMA per
2048-column superchunk (small interleaved stores measurably slow the V
read stream via HBM read/write turnarounds).

The host pre-arranges each core's V shard as [nchunk, 128, kc, 512] fp16
so every 512-column chunk is one 1 MB DMA whose per-partition source AND
SBUF destination runs are a contiguous 8 KB — near-ideal descriptors
(a 1 KB-run variant measured ~10% slower). Per-chunk DMA granularity
keeps tensor-engine waits in small quanta so the PE clock stays warm
(no >3.4 us HAM idle windows).

Regime: HBM-bound. V streams at the measured ~350 GB/s per-core cap;
67 MB / 350 GB/s ~= 193 us. Measured per-core exec 195-228 us
(mean ~206 us, spread is cross-core HBM/profiling contention),
vs 563 us baseline.
"""

import os
import sys

import numpy as np

for _p in ("/opt/trn_rl_repo", "/root/.axon_site/_ro/trn_rl_repo"):
    if os.path.isdir(_p) and _p not in sys.path:
        sys.path.insert(0, _p)

import concourse.bass as bass
import concourse.mybir as mybir
import concourse.tile as tile
from concourse import bacc
from concourse.bass_utils import run_bass_kernel_spmd

NCORES = 8
NN = 262144
IN_DIM = 1024
HIDDEN = 128
OUT_DIM = 4
R = NN // NCORES  # rows per core

P = 128            # SBUF partitions
KC = IN_DIM // P   # 8 k-chunks
CHUNK = 512        # columns per PSUM accumulation tile (one PSUM bank)
S = 4              # chunks per superchunk (live PSUM accumulator banks)
SGROUP = S * CHUNK # 2048 columns per DMA group / superchunk
DATA_BUFS = 4      # prefetch depth for V superchunk tiles

_last_results = None  # exposed for test harness (exec_time_ns etc.)
MODE = "f16"


def build_nc(rows=R):
    """Build the SPMD Bass program for one core."""
    f32 = mybir.dt.float32
    f16 = mybir.dt.float16

    nc = bacc.Bacc("TRN2")

    nch = rows // CHUNK
    vt_d = nc.declare_dram_parameter("VT", [nch, P, KC, CHUNK], f16, isOutput=False)
    w1_d = nc.declare_dram_parameter("W1T", [IN_DIM, HIDDEN], f16, isOutput=False)
    b1_d = nc.declare_dram_parameter("B1", [HIDDEN, 1], f32, isOutput=False)
    w2_d = nc.declare_dram_parameter("W2T", [HIDDEN, OUT_DIM], f16, isOutput=False)
    b2_d = nc.declare_dram_parameter("B2", [OUT_DIM, 1], f32, isOutput=False)
    out_d = nc.declare_dram_parameter("OUT", [OUT_DIM, rows], f32, isOutput=True)

    nsc = rows // SGROUP

    with tile.TileContext(nc) as tc:
        with (
            tc.tile_pool(name="const", bufs=1) as cpool,
            tc.tile_pool(name="data", bufs=DATA_BUFS) as dpool,
            tc.tile_pool(name="work", bufs=3) as wpool,
            tc.tile_pool(name="psum1", bufs=4, space="PSUM") as ppool,
            tc.tile_pool(name="psum2", bufs=2, space="PSUM") as opool,
        ):
            # --- constants ---
            # W1 goes first on the sync queue (first matmul needs it);
            # the small consts ride the scalar-engine HWDGE ring so they
            # don't delay the V stream on the sync ring.
            w1_sb = cpool.tile([P, KC, HIDDEN], f16)
            nc.sync.dma_start(w1_sb[:], w1_d[:].rearrange("(c p) h -> p c h", p=P))
            b1_sb = cpool.tile([HIDDEN, 1], f32)
            nc.scalar.dma_start(b1_sb[:], b1_d[:])
            w2_sb = cpool.tile([HIDDEN, OUT_DIM], f16)
            nc.scalar.dma_start(w2_sb[:], w2_d[:])
            b2_sb = cpool.tile([OUT_DIM, 1], f32)
            nc.scalar.dma_start(b2_sb[:], b2_d[:])

            vt_view = vt_d[:]
            out_sc_view = out_d[:].rearrange("o (g n) -> g o n", n=SGROUP)

            for g in range(nsc):
                v = dpool.tile([P, S, KC, CHUNK], f16, tag="v")
                # per-chunk 1 MB DMAs: source and SBUF destination are
                # both one contiguous 8 KB run per partition (ideal
                # descriptors), and PE waits come in small quanta so
                # the tensor engine never idles >3.4us (HAM stays warm)
                for j in range(S):
                    if g == 0:
                        # halve the first superchunk's transfers so the
                        # first matmul's data lands sooner
                        for h in range(2):
                            slk = slice(h * (KC // 2), (h + 1) * (KC // 2))
                            nc.sync.dma_start(
                                v[:, j, slk, :], vt_view[g * S + j][:, slk, :]
                            )
                    else:
                        nc.sync.dma_start(v[:, j, :, :], vt_view[g * S + j])

                # chunk-serial matmul order: the PE queue is in-order, so
                # a k-major interleave would pace every MM group on the
                # LAST-arriving chunk of the superchunk; chunk-serial lets
                # each chunk's 8 MMs (and its ACT/L2 chain) issue as soon
                # as that chunk's 1 MB DMA lands. LDWEIGHTS is hidden by
                # the PE reorder window either way.
                o_sb = wpool.tile([OUT_DIM, SGROUP], f32, tag="o")
                for j in range(S):
                    ps_j = ppool.tile([HIDDEN, CHUNK], f32, tag="ps")
                    for c in range(KC):
                        nc.tensor.matmul(
                            ps_j[:],
                            w1_sb[:, c, :],
                            v[:, j, c, :],
                            start=(c == 0),
                            stop=(c == KC - 1),
                        )
                    hh = wpool.tile([HIDDEN, CHUNK], f16, tag="hh")
                    nc.scalar.activation(
                        hh[:], ps_j[:],
                        mybir.ActivationFunctionType.Relu,
                        bias=b1_sb[:],
                    )
                    po = opool.tile([OUT_DIM, CHUNK], f32, tag="po")
                    nc.tensor.matmul(po[:], w2_sb[:], hh[:], start=True, stop=True)
                    nc.vector.tensor_scalar_add(
                        o_sb[:, j * CHUNK : (j + 1) * CHUNK], po[:], b2_sb[:]
                    )

                # one batched 32 KB store per superchunk: few HBM
                # read/write turnarounds (64 small stores measurably
                # slowed the V read stream)
                nc.scalar.dma_start(out_sc_view[g], o_sb[:])

    return nc


def kernel(V, W1, b1, W2, b2):
    global _last_results

    V = np.asarray(V, dtype=np.float32)
    W1 = np.asarray(W1, dtype=np.float32)
    b1 = np.asarray(b1, dtype=np.float32)
    W2 = np.asarray(W2, dtype=np.float32)
    b2 = np.asarray(b2, dtype=np.float32)

    common = {
        "W1T": W1.astype(np.float16),
        "W2T": np.ascontiguousarray(W2).astype(np.float16),
        "B1": np.ascontiguousarray(b1.reshape(HIDDEN, 1)),
        "B2": np.ascontiguousarray(b2.reshape(OUT_DIM, 1)),
    }

    in_maps = []
    for c in range(NCORES):
        shard = V[c * R : (c + 1) * R]  # [R, IN_DIM]
        # [nchunk, 512, kc, 128] -> [nchunk, 128, kc, 512]: each chunk's
        # per-partition data is one contiguous 8 KB run
        vh = np.ascontiguousarray(
            shard.reshape(R // CHUNK, CHUNK, KC, P)
            .transpose(0, 3, 2, 1)
            .astype(np.float16)
        )
        m = {"VT": vh}
        m.update(common)
        in_maps.append(m)

    nc = build_nc(R)
    nc.finalize()
    res = run_bass_kernel_spmd(nc, in_maps, list(range(NCORES)))
    _last_results = res

    out = np.concatenate(
        [np.asarray(r["OUT"]).T for r in res.results], axis=0
    ).astype(np.float32)
    return out


# revision 21
# speedup vs baseline: 1.0270x; 1.0028x over previous
"""Trainium2 Bass kernel for nn_Decoder (dense MLP).

Computes out = relu(V @ W1 + b1) @ W2 + b2 for V [262144, 1024],
W1 [1024, 128], W2 [128, 4].

Strategy
--------
Data-parallel over 8 NeuronCores: V is sharded along rows (32768 rows per
core); the small weights are replicated.

Single-pass fp16 (~4e-4 rel err, gate is 2e-2) instead of a multi-pass
hi/lo split — 3x less PE work; fp8 variants were measured at 2.9e-2+ and
fail the gate, so 2 B/elem is the DMA floor. Layer 1 computes
h.T = W1.T @ V.T via PSUM-accumulated matmuls, k-major over S=4 PSUM
banks so each W1 k-chunk's 128-col weight load is amortized over 4
matmuls. ReLU(+b1) on the scalar engine reads PSUM and emits fp16; layer
2 (out.T = W2.T @ h.T) runs on the tensor engine; b2 is added on the
vector engine and results are stored in one batched 32 KB DMA per
2048-column superchunk (small interleaved stores measurably slow the V
read stream via HBM read/write turnarounds).

The host pre-arranges each core's V shard as [nchunk, 128, kc, 512] fp16
so every 512-column chunk is one 1 MB DMA whose per-partition source AND
SBUF destination runs are a contiguous 8 KB — near-ideal descriptors
(a 1 KB-run variant measured ~10% slower). Per-chunk DMA granularity
keeps tensor-engine waits in small quanta so the PE clock stays warm
(no >3.4 us HAM idle windows).

Regime: HBM-bound. V streams at the measured ~350 GB/s per-core cap;
67 MB / 350 GB/s ~= 193 us. Measured per-core exec 195-228 us
(mean ~206 us, spread is cross-core HBM/profiling contention),
vs 563 us baseline.
"""

import os
import sys

import numpy as np

for _p in ("/opt/trn_rl_repo", "/root/.axon_site/_ro/trn_rl_repo"):
    if os.path.isdir(_p) and _p not in sys.path:
        sys.path.insert(0, _p)

import concourse.bass as bass
import concourse.mybir as mybir
import concourse.tile as tile
from concourse import bacc
from concourse.bass_utils import run_bass_kernel_spmd

NCORES = 8
NN = 262144
IN_DIM = 1024
HIDDEN = 128
OUT_DIM = 4
R = NN // NCORES  # rows per core

P = 128            # SBUF partitions
KC = IN_DIM // P   # 8 k-chunks
CHUNK = 512        # columns per PSUM accumulation tile (one PSUM bank)
S = 4              # chunks per superchunk (live PSUM accumulator banks)
SGROUP = S * CHUNK # 2048 columns per DMA group / superchunk
DATA_BUFS = 4      # prefetch depth for V superchunk tiles

_last_results = None  # exposed for test harness (exec_time_ns etc.)
MODE = "f16"


def build_nc(rows=R):
    """Build the SPMD Bass program for one core."""
    f32 = mybir.dt.float32
    f16 = mybir.dt.float16

    nc = bacc.Bacc("TRN2")

    nch = rows // CHUNK
    vt_d = nc.declare_dram_parameter("VT", [nch, P, KC, CHUNK], f16, isOutput=False)
    w1_d = nc.declare_dram_parameter("W1T", [IN_DIM, HIDDEN], f16, isOutput=False)
    b1_d = nc.declare_dram_parameter("B1", [HIDDEN, 1], f32, isOutput=False)
    w2_d = nc.declare_dram_parameter("W2T", [HIDDEN, OUT_DIM], f16, isOutput=False)
    b2_d = nc.declare_dram_parameter("B2", [OUT_DIM, 1], f32, isOutput=False)
    out_d = nc.declare_dram_parameter("OUT", [OUT_DIM, rows], f32, isOutput=True)

    nsc = rows // SGROUP

    with tile.TileContext(nc) as tc:
        with (
            tc.tile_pool(name="const", bufs=1) as cpool,
            tc.tile_pool(name="data", bufs=DATA_BUFS) as dpool,
            tc.tile_pool(name="work", bufs=3) as wpool,
            tc.tile_pool(name="psum1", bufs=4, space="PSUM") as ppool,
            tc.tile_pool(name="psum2", bufs=2, space="PSUM") as opool,
        ):
            # --- constants ---
            # W1 goes first on the sync queue (first matmul needs it);
            # the small consts ride the scalar-engine HWDGE ring so they
            # don't delay the V stream on the sync ring.
            w1_sb = cpool.tile([P, KC, HIDDEN], f16)
            nc.sync.dma_start(w1_sb[:], w1_d[:].rearrange("(c p) h -> p c h", p=P))
            b1_sb = cpool.tile([HIDDEN, 1], f32)
            nc.scalar.dma_start(b1_sb[:], b1_d[:])
            w2_sb = cpool.tile([HIDDEN, OUT_DIM], f16)
            nc.scalar.dma_start(w2_sb[:], w2_d[:])
            b2_sb = cpool.tile([OUT_DIM, 1], f32)
            nc.scalar.dma_start(b2_sb[:], b2_d[:])

            vt_view = vt_d[:]
            out_sc_view = out_d[:].rearrange("o (g n) -> g o n", n=SGROUP)

            for g in range(nsc):
                v = dpool.tile([P, S, KC, CHUNK], f16, tag="v")
                # per-chunk 1 MB DMAs: source and SBUF destination are
                # both one contiguous 8 KB run per partition (ideal
                # descriptors), and PE waits come in small quanta so
                # the tensor engine never idles >3.4us (HAM stays warm)
                for j in range(S):
                    if g == 0:
                        # halve the first superchunk's transfers so the
                        # first matmul's data lands sooner
                        for h in range(2):
                            slk = slice(h * (KC // 2), (h + 1) * (KC // 2))
                            nc.sync.dma_start(
                                v[:, j, slk, :], vt_view[g * S + j][:, slk, :]
                            )
                    else:
                        nc.sync.dma_start(v[:, j, :, :], vt_view[g * S + j])

                # layer 1: k-major so each W1 k-chunk stays stationary
                # across S matmuls (amortizes the 128-col weight load)
                ps = [
                    ppool.tile([HIDDEN, CHUNK], f32, tag="ps", name=f"ps{j}")
                    for j in range(S)
                ]
                for c in range(KC):
                    for j in range(S):
                        nc.tensor.matmul(
                            ps[j][:],
                            w1_sb[:, c, :],
                            v[:, j, c, :],
                            start=(c == 0),
                            stop=(c == KC - 1),
                        )

                o_sb = wpool.tile([OUT_DIM, SGROUP], f32, tag="o")
                for j in range(S):
                    hh = wpool.tile([HIDDEN, CHUNK], f16, tag="hh")
                    nc.scalar.activation(
                        hh[:], ps[j][:],
                        mybir.ActivationFunctionType.Relu,
                        bias=b1_sb[:],
                    )
                    po = opool.tile([OUT_DIM, CHUNK], f32, tag="po")
                    nc.tensor.matmul(po[:], w2_sb[:], hh[:], start=True, stop=True)
                    nc.vector.tensor_scalar_add(
                        o_sb[:, j * CHUNK : (j + 1) * CHUNK], po[:], b2_sb[:]
                    )

                # one batched 32 KB store per superchunk: few HBM
                # read/write turnarounds (64 small stores measurably
                # slowed the V read stream)
                nc.scalar.dma_start(out_sc_view[g], o_sb[:])

    return nc


def kernel(V, W1, b1, W2, b2):
    global _last_results

    V = np.asarray(V, dtype=np.float32)
    W1 = np.asarray(W1, dtype=np.float32)
    b1 = np.asarray(b1, dtype=np.float32)
    W2 = np.asarray(W2, dtype=np.float32)
    b2 = np.asarray(b2, dtype=np.float32)

    common = {
        "W1T": W1.astype(np.float16),
        "W2T": np.ascontiguousarray(W2).astype(np.float16),
        "B1": np.ascontiguousarray(b1.reshape(HIDDEN, 1)),
        "B2": np.ascontiguousarray(b2.reshape(OUT_DIM, 1)),
    }

    in_maps = []
    for c in range(NCORES):
        shard = V[c * R : (c + 1) * R]  # [R, IN_DIM]
        # [nchunk, 512, kc, 128] -> [nchunk, 128, kc, 512]: each chunk's
        # per-partition data is one contiguous 8 KB run
        vh = np.ascontiguousarray(
            shard.reshape(R // CHUNK, CHUNK, KC, P)
            .transpose(0, 3, 2, 1)
            .astype(np.float16)
        )
        m = {"VT": vh}
        m.update(common)
        in_maps.append(m)

    nc = build_nc(R)
    nc.finalize()
    res = run_bass_kernel_spmd(nc, in_maps, list(range(NCORES)))
    _last_results = res

    out = np.concatenate(
        [np.asarray(r["OUT"]).T for r in res.results], axis=0
    ).astype(np.float32)
    return out
